# revision 1
# baseline (speedup 1.0000x reference)
"""BinaryConv2D Trainium2 kernel.

Reference computation:
    out = conv2d(sign(x), sign(w), SAME, stride 1)   # sign(v) = +1 if v>=0 else -1
    x: (64, 56, 56, 128) f32, w: (3, 3, 128, 256) f32 -> out (64, 56, 56, 256) f32

Strategy (data-parallel over batch, 8 images per NeuronCore):
  1. SWDGE cast-DMA x f32 -> bf16 (HBM->HBM), 2 images per DMA.  The cast
     preserves sign, and only the sign bit is consumed downstream.
  2. Per image pair: HW xbar DMA-transpose (DRAM->SBUF) [6272 px, 128 ch] ->
     [128 ch, 6272 px] bf16.  Weights are binarized host-side and loaded with
     another xbar transpose.
  3. One DVE tensor_scalar op per image binarizes via bit ops on the bf16
     pattern ((v & 0x8000) | 0x3F80 -> exactly +-1.0) while scattering rows
     into a zero-padded 58x58 layout (SAME padding becomes pointer shifts).
  4. 3x3 conv = 9 accumulating matmuls per output tile.  Output stays
     pixel-major: out[px, co] = sum_taps xpad[ci, px+s].T @ w_tap[ci, co]
     with lhsT (stationary) = x tile [128ci x 116px] (2 padded rows), rhs =
     w tap [128ci x 256co], PSUM f32 [116 x 256].  All values are +-1 in
     bf16, accumulation is f32 -> arithmetic is exact.
  5. DVE copies PSUM -> SBUF stage; two large DMAs per half-image write the
     NHWC output (even rows / odd rows) back to HBM.

Built on bacc.Bacc (not raw Bass) so multi-semaphore waits are legalized
into EventSemaphore chains (TRN2 instructions hold at most one sync wait).
"""

import sys

if "/opt/trn_rl_repo" not in sys.path:
    sys.path.insert(0, "/opt/trn_rl_repo")

import numpy as np

import concourse.bacc as bacc
import concourse.bass as bass
import concourse.mybir as mybir
from concourse.tile import TileContext
from concourse.bass_utils import run_bass_kernel_spmd

N_CORES = 8
IMGS = 8  # images per core
H = W = 56
C = 128  # input channels (= contraction dim = SBUF partitions)
O = 256  # output channels
PW = 58  # padded row width
PH = 58  # padded rows per image (rows 0 and 57 are the SAME-padding rows)
PPI = PH * PW  # padded pixels per image (3364)
GUARD_L = 1  # zero guard before image 0 (tap offset -59 at tile 0)
GUARD_R = 4
TILES = H // 2  # 28 output tiles per image, 2 output rows each
F32 = mybir.dt.float32
BF16 = mybir.dt.bfloat16
U16 = mybir.dt.uint16

# tap order k = 3*di + dj ; shift in padded flat coords
TAP_SHIFTS = [PW * (di - 1) + (dj - 1) for di in range(3) for dj in range(3)]


def build_nc() -> bass.Bass:
    nc = bacc.Bacc()
    x_t = nc.dram_tensor("x", [IMGS, H, W, C], F32, kind="ExternalInput")
    # host-binarized weights, laid out [tap*co, ci] so one xbar DMA-transpose
    # loads them as [ci, tap*co]
    wbt_t = nc.dram_tensor("wbt", [9 * O, C], BF16, kind="ExternalInput")
    y_t = nc.dram_tensor("out", [IMGS, H, W, O], F32, kind="ExternalOutput")
    # per-pair bf16 bounce tensors keep DRAM dependency tracking precise
    xb_ts = [
        nc.dram_tensor(f"xb{p}", [2 * H * W, C], BF16) for p in range(IMGS // 2)
    ]

    with TileContext(nc) as tc:
        with (
            tc.tile_pool(name="const", bufs=1) as constp,
            tc.tile_pool(name="xtr", bufs=IMGS // 2) as xtrp,
            tc.tile_pool(name="stage", bufs=3) as stagep,
            tc.tile_pool(name="psum", bufs=6, space="PSUM") as psump,
        ):
            # ---- weights: single xbar transpose load of host-binarized w ----
            wb = constp.tile([C, 9 * O], BF16)
            nc.sync.dma_start(out=wb[:], in_=wbt_t[:], transpose=True)

            # ---- per-image zero-padded, channel-major input planes ----
            # Zero only the padding ranges (disjoint from the binarize write
            # range) to keep the dependency structure lean.
            xpads = []
            for i in range(IMGS):
                xp = constp.tile([C, GUARD_L + PPI + GUARD_R], BF16, tag=f"xpad{i}")
                # head: guard + top pad row + col0 of data row 1 -> [0, 60)
                nc.vector.memset(xp[:, 0:60], 0.0)
                # interior: col57 of row r + col0 of row r+1 -> [58k, 58k+2)
                nc.vector.memset(
                    xp[:, 116 : 116 + 55 * PW].rearrange("c (r w) -> c r w", w=PW)[
                        :, :, 0:2
                    ],
                    0.0,
                )
                # tail: col57 of row 56 + bottom pad row + guard
                nc.vector.memset(xp[:, 3306 : GUARD_L + PPI + GUARD_R], 0.0)
                xpads.append(xp)

            # ---- input pipeline: cast pairs, transpose pairs ----
            xtrs = {}
            for p in range(IMGS // 2):
                nc.gpsimd.dma_start(
                    out=xb_ts[p][:],
                    in_=x_t[2 * p : 2 * p + 2].rearrange("n h w c -> (n h w) c"),
                )
                xtr = xtrp.tile([C, 2 * H * W], BF16)
                nc.sync.dma_start(out=xtr[:], in_=xb_ts[p][:], transpose=True)
                xtrs[p] = xtr

            for i in range(IMGS):
                xtr = xtrs[i // 2]
                xoff = (i % 2) * H * W
                # binarize + scatter into padded rows (56 rows, stride 58)
                s0 = GUARD_L + PW + 1
                dst = xpads[i][:, s0 : s0 + H * PW].rearrange(
                    "c (r w) -> c r w", w=PW
                )[:, :, 0:W]
                src = xtr[:, xoff : xoff + H * W].rearrange("c (r w) -> c r w", w=W)
                nc.vector.tensor_scalar(
                    dst.bitcast(U16),
                    src.bitcast(U16),
                    0x8000,
                    0x3F80,
                    op0=mybir.AluOpType.bitwise_and,
                    op1=mybir.AluOpType.bitwise_or,
                )

                # ---- 28 output tiles (2 rows each) of 9 accumulating matmuls,
                # staged in half-image chunks of 14 tiles to bound SBUF ----
                HT = TILES // 2  # 14
                for half in range(2):
                    stage = stagep.tile([128, HT * O], F32)
                    st3 = stage[:].rearrange("p (t o) -> p t o", o=O)
                    for th in range(HT):
                        t = half * HT + th
                        ps = psump.tile([128, O], F32)
                        p0 = GUARD_L + PW * (1 + 2 * t)  # padded start of tile
                        for k, s in enumerate(TAP_SHIFTS):
                            a = p0 + s
                            nc.tensor.matmul(
                                ps[:116, :],
                                xpads[i][:, a : a + 116],
                                wb[:, k * O : (k + 1) * O],
                                start=(k == 0),
                                stop=(k == 8),
                            )
                        nc.vector.tensor_copy(
                            stage[:116, th * O : (th + 1) * O], ps[:116, :]
                        )

                    # ---- write out: partitions 1..56 = even rows, 59..114 odd
                    rows = y_t[i][half * 2 * HT : (half + 1) * 2 * HT]
                    ye = rows.rearrange("(r2 two) w c -> two w r2 c", two=2)
                    nc.gpsimd.dma_start(out=ye[0], in_=st3[1 : 1 + W])
                    nc.gpsimd.dma_start(out=ye[1], in_=st3[59 : 59 + W])

    nc.finalize()
    return nc


_NC_CACHE = None


def _get_nc():
    global _NC_CACHE
    if _NC_CACHE is None:
        _NC_CACHE = build_nc()
    return _NC_CACHE


def prep_wbt(w: np.ndarray) -> np.ndarray:
    """Binarize + transpose weights on host: (3,3,128,256) f32 ->
    [9*256, 128] bf16 with exact +-1 values (replicated to every core)."""
    import ml_dtypes

    wb = np.where(w >= 0, np.float32(1.0), np.float32(-1.0))
    # [di, dj, ci, co] -> [(di dj) co, ci]
    wbt = wb.transpose(0, 1, 3, 2).reshape(9 * O, C)
    return np.ascontiguousarray(wbt.astype(ml_dtypes.bfloat16))


def _ntff_hook():
    """NTFF capture context manager via the axon PJRT .so (the installed
    antenv lacks axon_hooks, so build the ctypes hook directly)."""
    sys.path.insert(0, "/root/.axon_site")
    from trn_agent_boot.trn_boot import _ntff_profile_via_ctypes

    return _ntff_profile_via_ctypes("/opt/axon/libaxon_pjrt.so")


def run(inputs: dict, profile_dir: str | None = None):
    """Run on all 8 NeuronCores. Returns (full_output, BassKernelResults)."""
    x = np.ascontiguousarray(np.asarray(inputs["x"], dtype=np.float32))
    w = np.ascontiguousarray(np.asarray(inputs["w"], dtype=np.float32))
    assert x.shape == (N_CORES * IMGS, H, W, C), x.shape
    assert w.shape == (3, 3, C, O), w.shape

    nc = _get_nc()
    wbt = prep_wbt(w)
    in_maps = [
        {"x": x[i * IMGS : (i + 1) * IMGS], "wbt": wbt} for i in range(N_CORES)
    ]
    if profile_dir is not None:
        hook = _ntff_hook()
        with hook(profile_dir, [0]):
            res = run_bass_kernel_spmd(nc, in_maps, list(range(N_CORES)))
    else:
        res = run_bass_kernel_spmd(nc, in_maps, list(range(N_CORES)))
    out = np.concatenate([res.results[i]["out"] for i in range(N_CORES)], axis=0)
    return out, res


def kernel(**inputs: np.ndarray) -> np.ndarray:
    out, _ = run(inputs)
    return out



# revision 3
# speedup vs baseline: 1.5192x; 1.5192x over previous
"""BinaryConv2D Trainium2 kernel (fp8 DoubleRow, weight-stationary).

Reference computation:
    out = conv2d(sign(x), sign(w), SAME, stride 1)   # sign(v) = +1 if v>=0 else -1
    x: (64, 56, 56, 128) f32, w: (3, 3, 128, 256) f32 -> out (64, 56, 56, 256) f32

Strategy (data-parallel over batch, 8 images per NeuronCore; per-core output
is produced channel-major and the host gather re-interleaves to NHWC):
  1. SWDGE cast-DMA x f32 -> bf16 (HBM->HBM) per image, then HW xbar
     DMA-transpose (DRAM->SBUF) [3136 px, 128 ch] -> [128 ch, 3136 px].
  2. One DVE tensor_scalar per image maps x to +-0.5 in fp8e4
     ((v >= 0) - 0.5) while scattering rows into a zero-padded 58x58 plane
     (SAME padding becomes pointer shifts).  Weights are host-binarized to
     +-1 fp8e4; the overall x2 scale is folded into the PSUM drain.
  3. Conv contracts 9 taps x 128 ci.  Taps are processed in 4 pairs via
     fp8 DoubleRow matmuls (2 MACs/cell/cycle, contraction 256) plus one
     normal fp8 matmul, weight-stationary: lhsT = w[128ci, (2tap), 128co],
     moving rhs = two tap-shifted x windows [128ci, (2, N)] streamed from
     the padded plane, accumulating PSUM [128co, N<=512px] in f32 (exact).
  4. PSUM tiles are drained with a x2 scale to bf16 (alternating ScalarE /
     VectorE so neither engine is a bottleneck) and written to HBM as
     [img, co_half, 128co, 3248 px-run]; the host strips the padding
     columns and transposes to NHWC f32 (error ~2^-9, integer output).

Built on bacc.Bacc so multi-semaphore waits are legalized into
EventSemaphore chains.
"""

import sys

if "/opt/trn_rl_repo" not in sys.path:
    sys.path.insert(0, "/opt/trn_rl_repo")

import numpy as np

import concourse.bacc as bacc
import concourse.bass as bass
import concourse.mybir as mybir
from concourse.bass import AP
from concourse.tile import TileContext
from concourse.bass_utils import run_bass_kernel_spmd

N_CORES = 8
IMGS = 8  # images per core
H = W = 56
C = 128  # input channels (= contraction dim = SBUF partitions)
O = 256  # output channels
PW = 58  # padded row width
PPI = PW * PW  # padded pixels per image (3364), rows 0/57 + cols 0/57 are pad
GUARD = 59  # zero guard on both sides of the padded plane (max |tap shift|)
XPW = GUARD + PPI + GUARD  # SBUF padded-plane width (3482)
RUN = PW * H  # contiguous output px run [row1..row56] = 3248
NT = 512  # px per PSUM tile
TILES = (RUN + NT - 1) // NT  # 7 (6x512 + 176)
F32 = mybir.dt.float32
BF16 = mybir.dt.bfloat16
FP8 = mybir.dt.float8e4

# tap order k = 3*di + dj ; shift in padded flat coords
TAP_SHIFTS = [PW * (di - 1) + (dj - 1) for di in range(3) for dj in range(3)]
# 4 DoubleRow pairs (taps 2p, 2p+1) + single tap 8
PAIR_S0 = [TAP_SHIFTS[2 * p] for p in range(4)]
PAIR_DS = [TAP_SHIFTS[2 * p + 1] - TAP_SHIFTS[2 * p] for p in range(4)]
S8 = TAP_SHIFTS[8]


def build_nc() -> bass.Bass:
    nc = bacc.Bacc()
    x_t = nc.dram_tensor("x", [IMGS, H * W, C], F32, kind="ExternalInput")
    # host-binarized weights [ci, 4*(2tap x 256co) + 256co] fp8e4
    wq_t = nc.dram_tensor("wq", [C, 9 * O], FP8, kind="ExternalInput")
    y_t = nc.dram_tensor("out", [IMGS, 2, C, RUN], BF16, kind="ExternalOutput")
    xb_ts = [nc.dram_tensor(f"xb{i}", [H * W, C], BF16) for i in range(IMGS)]

    with TileContext(nc) as tc:
        with (
            tc.tile_pool(name="const", bufs=1) as constp,
            tc.tile_pool(name="xtr", bufs=3) as xtrp,
            tc.tile_pool(name="xpad", bufs=3) as xpadp,
            tc.tile_pool(name="ostage", bufs=3) as ostagep,
            tc.tile_pool(name="psum", bufs=8, space="PSUM") as psump,
        ):
            wt = constp.tile([C, 9 * O], FP8)
            nc.sync.dma_start(out=wt[:], in_=wq_t[:])

            def w_pair_ap(p: int, h: int) -> AP:
                # [ci, (2 taps), (128 co)] slice of the pair-p block
                return wt[:, 512 * p : 512 * p + 512].rearrange(
                    "c (two co) -> c two co", two=2
                )[:, :, 128 * h : 128 * h + 128]

            for i in range(IMGS):
                nc.gpsimd.dma_start(out=xb_ts[i][:], in_=x_t[i])  # f32 -> bf16
                xtr = xtrp.tile([C, H * W], BF16)
                nc.sync.dma_start(out=xtr[:], in_=xb_ts[i][:], transpose=True)

                xp = xpadp.tile([C, XPW], FP8)
                # zero the guards + SAME-padding ring (disjoint from data)
                nc.vector.memset(xp[:, 0 : GUARD + PW + 1], 0.0)
                nc.vector.memset(
                    xp[:, GUARD + 2 * PW - 1 : GUARD + 2 * PW - 1 + 55 * PW]
                    .rearrange("c (r w) -> c r w", w=PW)[:, :, 0:2],
                    0.0,
                )
                nc.vector.memset(xp[:, GUARD + PPI - PW - 1 : XPW], 0.0)
                # binarize to +-0.5 fp8, scattering 56-px rows into the plane
                dst = xp[:, GUARD + PW + 1 : GUARD + PW + 1 + H * PW].rearrange(
                    "c (r w) -> c r w", w=PW
                )[:, :, 0:W]
                src = xtr[:].rearrange("c (r w) -> c r w", w=W)
                nc.vector.tensor_scalar(
                    dst,
                    src,
                    0.0,
                    0.5,
                    op0=mybir.AluOpType.is_ge,
                    op1=mybir.AluOpType.subtract,
                )

                # out px j in [0, RUN) is padded-plane px p = j + PW + 1
                # (j = 58*(r-1) + (w-1) for row r, col w) -> SBUF idx j + base
                base = GUARD + PW + 1
                for h in range(2):
                    ostage = ostagep.tile([C, RUN], BF16)
                    for t in range(TILES):
                        p0 = NT * t
                        n = min(NT, RUN - p0)
                        ps = psump.tile([C, NT], F32)
                        for p in range(4):
                            rhs = AP(
                                xp.tensor,
                                xp[:, 0:1].offset + base + p0 + PAIR_S0[p],
                                [[XPW, C], [PAIR_DS[p], 2], [1, n]],
                            )
                            nc.tensor.matmul(
                                ps[:, :n],
                                w_pair_ap(p, h),
                                rhs,
                                start=(p == 0),
                                stop=False,
                                perf_mode=mybir.MatmulPerfMode.DoubleRow,
                            )
                        a8 = base + p0 + S8
                        nc.tensor.matmul(
                            ps[:, :n],
                            wt[:, 8 * O + 128 * h : 8 * O + 128 * h + 128],
                            xp[:, a8 : a8 + n],
                            start=False,
                            stop=True,
                        )
                        # drain with the x2 binarization scale, alternating
                        # engines (ScalarE sits closer to PSUM)
                        if t % 2 == 0:
                            nc.scalar.mul(ostage[:, p0 : p0 + n], ps[:, :n], 2.0)
                        else:
                            nc.vector.tensor_scalar_mul(
                                ostage[:, p0 : p0 + n], ps[:, :n], 2.0
                            )
                    nc.sync.dma_start(out=y_t[i][h], in_=ostage[:])

    nc.finalize()
    return nc


_NC_CACHE = None


def _get_nc():
    global _NC_CACHE
    if _NC_CACHE is None:
        _NC_CACHE = build_nc()
    return _NC_CACHE


def prep_wq(w: np.ndarray) -> np.ndarray:
    """Binarize + lay out weights on host: (3,3,128,256) f32 ->
    [128ci, 4 pair-blocks of (2 taps x 256 co) + 256 co] fp8e4 +-1."""
    import ml_dtypes

    wb = np.where(w >= 0, np.float32(1.0), np.float32(-1.0))
    # [di, dj, ci, co] -> [tap, ci, co]
    taps = wb.reshape(9, C, O)
    wq = np.empty((C, 9 * O), dtype=np.float32)
    for p in range(4):
        wq[:, 512 * p : 512 * p + 256] = taps[2 * p]
        wq[:, 512 * p + 256 : 512 * p + 512] = taps[2 * p + 1]
    wq[:, 8 * O : 9 * O] = taps[8]
    return np.ascontiguousarray(wq.astype(ml_dtypes.float8_e4m3))


def _ntff_hook():
    """NTFF capture context manager via the axon PJRT .so (the installed
    antenv lacks axon_hooks, so build the ctypes hook directly)."""
    sys.path.insert(0, "/root/.axon_site")
    from trn_agent_boot.trn_boot import _ntff_profile_via_ctypes

    return _ntff_profile_via_ctypes("/opt/axon/libaxon_pjrt.so")


def run(inputs: dict, profile_dir: str | None = None):
    """Run on all 8 NeuronCores. Returns (full_output, BassKernelResults)."""
    x = np.ascontiguousarray(np.asarray(inputs["x"], dtype=np.float32))
    w = np.ascontiguousarray(np.asarray(inputs["w"], dtype=np.float32))
    assert x.shape == (N_CORES * IMGS, H, W, C), x.shape
    assert w.shape == (3, 3, C, O), w.shape

    nc = _get_nc()
    wq = prep_wq(w)
    xr = x.reshape(N_CORES, IMGS, H * W, C)
    in_maps = [{"x": xr[i], "wq": wq} for i in range(N_CORES)]
    if profile_dir is not None:
        hook = _ntff_hook()
        with hook(profile_dir, [0]):
            res = run_bass_kernel_spmd(nc, in_maps, list(range(N_CORES)))
    else:
        res = run_bass_kernel_spmd(nc, in_maps, list(range(N_CORES)))

    # device layout [img, co_half, 128co, 3248] -> NHWC f32
    out = np.empty((N_CORES * IMGS, H, W, O), dtype=np.float32)
    for c in range(N_CORES):
        yq = np.asarray(res.results[c]["out"]).astype(np.float32)
        v = yq.reshape(IMGS, 2, C, H, PW)[..., :W]  # strip pad cols
        out[c * IMGS : (c + 1) * IMGS] = v.transpose(0, 3, 4, 1, 2).reshape(
            IMGS, H, W, O
        )
    return out, res


def kernel(**inputs: np.ndarray) -> np.ndarray:
    out, _ = run(inputs)
    return out
